# revision 28
# baseline (speedup 1.0000x reference)
"""Trainium2 Bass kernel for the two-layer GCN (node layer -> edge layer -> log_softmax).

Math (reference):
  d1 = Z @ p1
  M1 = (T * d1) @ T.T                      (symmetric, [Nv, Nv])
  A1 = M1 * adj_v, diag replaced by diag(adj_v)
  X1 = relu(A1 @ (X @ W1) + b1);  Z1 = relu(Z)
  d2 = X1 @ p2
  M2 = (T.T * d2) @ T                      (symmetric, [Ne, Ne])
  A2 = M2 * adj_e, diag replaced by diag(adj_e)
  Zout = A2 @ (Z1 @ W2) + b2
  out  = log_softmax(Zout, axis=0)

Sharding (8 cores): core c owns output rows Nv-shard (layer 1) and Ne-shard
(layer 2).  The mult matmuls are computed in transposed block layout
P[j, i] = M[i, j] (exploiting symmetry of M) so that no on-device transposes
are needed anywhere:
  layer 1: lhsT = Tt[:, j-tile] (Tt = T.T streamed),  rhs = d1-scaled Tt[:, i-shard]
  layer 2: lhsT = T[:, j-tile]  (T streamed),         rhs = d2-scaled T[:, i-shard]
The elementwise adjacency mask uses host-transposed adjacency shards
(diagonal zeroed; the diagonal term diag(adj) @ Hw is added separately as a
row-scale of Hw).  The SpMM consumes the [j, i] tiles directly as the matmul
stationary operand.  d2 requires one tiny AllGather; log_softmax over the
sharded axis uses two [8]-element AllReduces (max, add).

All heavy matmuls run in bf16 (f32 PSUM accumulation); measured end-to-end
relative error vs the f32 reference is ~3e-3.
"""

import numpy as np
import ml_dtypes

BF16 = np.dtype(ml_dtypes.bfloat16)

# Full problem config
NV, NE = 4096, 8192
NFV, NFE, NHID, NCLASS = 128, 64, 128, 8
N_CORES = 8

# Debug: 0 = no collectives (garbage numerics), 1 = AllGather only, 2 = all
COLL_MODE = 2


def _dt():
    import concourse.mybir as mybir
    return mybir


def build_program(nv=NV, ne=NE, n_cores=N_CORES, stage=99, repeat=1, diag='full'):
    """repeat>1 builds a timing variant: whole body in a For_i loop,
    collectives skipped (set COLL_MODE=0), d-scales replaced by *1.0."""
    """Build the SPMD single-core program (same for every core)."""
    import concourse.bacc as bacc
    import concourse.mybir as mybir
    import concourse.tile as tile

    f32 = mybir.dt.float32
    bf16 = mybir.dt.bfloat16
    MUL = mybir.AluOpType.mult
    ADD = mybir.AluOpType.add
    X_AX = mybir.AxisListType.X
    ACT = mybir.ActivationFunctionType

    iv = nv // n_cores            # vertex rows per core  (512)
    ie = ne // n_cores            # edge rows per core    (1024)
    assert iv % 128 == 0 and ie % 512 == 0
    vt_n = nv // 128              # v-tiles total         (32)
    et_n = ne // 128              # e-tiles total         (64)
    ivt = iv // 128               # v-tiles per core      (4)
    ieh = ie // 512               # 512-wide moving halves in L2 (2)

    nc = bacc.Bacc(None, num_devices=n_cores)

    # ---- I/O ----
    tt_full = nc.dram_tensor("tt_full", [ne, nv], bf16, kind="ExternalInput")
    t_full = nc.dram_tensor("t_full", [nv, ne], bf16, kind="ExternalInput")
    tt_mov = nc.dram_tensor("tt_mov", [ne, iv], bf16, kind="ExternalInput")
    t_mov = nc.dram_tensor("t_mov", [nv, ie], bf16, kind="ExternalInput")
    advt = nc.dram_tensor("advt", [nv, iv], f32, kind="ExternalInput")
    aevt = nc.dram_tensor("aevt", [ne, ie], f32, kind="ExternalInput")
    avdiag = nc.dram_tensor("avdiag", [iv, 1], f32, kind="ExternalInput")
    aedb = nc.dram_tensor("aedb", [NCLASS, ie], f32, kind="ExternalInput")
    xt = nc.dram_tensor("xt", [NFV, nv], f32, kind="ExternalInput")
    xt_sh = nc.dram_tensor("xt_sh", [NFV, iv], f32, kind="ExternalInput")
    w1 = nc.dram_tensor("w1", [NFV, NHID], f32, kind="ExternalInput")
    b1b = nc.dram_tensor("b1b", [128, NHID], f32, kind="ExternalInput")
    zin = nc.dram_tensor("zin", [ne, NFE], f32, kind="ExternalInput")
    p1b = nc.dram_tensor("p1b", [128, NFE], f32, kind="ExternalInput")
    p2b = nc.dram_tensor("p2b", [128, NHID], f32, kind="ExternalInput")
    ztf = nc.dram_tensor("ztf", [NFE, ne], f32, kind="ExternalInput")
    zt_sh = nc.dram_tensor("zt_sh", [NFE, ie], f32, kind="ExternalInput")
    w2 = nc.dram_tensor("w2", [NFE, NCLASS], f32, kind="ExternalInput")
    b2c = nc.dram_tensor("b2c", [NCLASS, 1], f32, kind="ExternalInput")
    zt_out = nc.dram_tensor("zt_out", [NCLASS, ie], f32, kind="ExternalOutput")

    with tile.TileContext(nc) as tc:
        with (
            tc.tile_pool(name="mov", bufs=1) as movp,          # 64KB/p resident moving operand
            tc.tile_pool(name="slab", bufs=2) as slabp,        # streamed stationary slabs
            tc.tile_pool(name="adj", bufs=3) as adjp,          # streamed adjacency tiles
            tc.tile_pool(name="abuf", bufs=5) as abufp,        # A-tiles (bf16)
            tc.tile_pool(name="hw", bufs=1) as hwp,            # Hw1 / Hw2 / HwS residents
            tc.tile_pool(name="small", bufs=1) as smallp,      # weights, d-vectors, etc.
            tc.tile_pool(name="stream", bufs=2) as streamp,    # prologue streams
            tc.tile_pool(name="x1", bufs=2) as x1p,
            tc.tile_pool(name="zb", bufs=1) as zbp,
            tc.tile_pool(name="mult_ps", bufs=4, space="PSUM") as multps,
            tc.tile_pool(name="acc_ps", bufs=4, space="PSUM") as accps,
            tc.tile_pool(name="dram", bufs=1, space="DRAM") as dramp,
        ):
          import contextlib
          _lp = contextlib.ExitStack()
          if repeat > 1:
              _lp.enter_context(tc.For_i(0, repeat, 1))
          with _lp:
            # ---------- prologue: constants ----------
            w1_t = smallp.tile([NFV, NHID], f32, tag="w1")
            nc.sync.dma_start(w1_t[:], w1[:])
            b1b_t = smallp.tile([128, NHID], f32, tag="b1b")
            nc.sync.dma_start(b1b_t[:], b1b[:])
            p1b_t = smallp.tile([128, NFE], f32, tag="p1b")
            nc.sync.dma_start(p1b_t[:], p1b[:])
            p1rep_t = smallp.tile([128, 16 * NFE], f32, tag="p1rep")
            nc.vector.tensor_copy(p1rep_t[:, 0:NFE], p1b_t[:])
            _w = NFE
            while _w < 16 * NFE:
                nc.vector.tensor_copy(p1rep_t[:, _w:2 * _w], p1rep_t[:, 0:_w])
                _w *= 2
            p2b_t = smallp.tile([128, NHID], f32, tag="p2b")
            nc.sync.dma_start(p2b_t[:], p2b[:])
            w2_t = smallp.tile([NFE, NCLASS], f32, tag="w2")
            nc.sync.dma_start(w2_t[:], w2[:])
            b2_t = smallp.tile([NCLASS, 1], f32, tag="b2")
            nc.sync.dma_start(b2_t[:], b2c[:])
            avd_t = smallp.tile([128, ivt], f32, tag="avd")
            nc.sync.dma_start(
                avd_t[:], avdiag[:].rearrange("(t p) o -> p (t o)", p=128)
            )
            aedb_t = smallp.tile([NCLASS, ie], f32, tag="aedb")
            nc.sync.dma_start(aedb_t[:], aedb[:])

            # ---------- Hw1 = X @ W1  (bf16, [v-part tiles, h]) ----------
            hw1 = hwp.tile([128, vt_n * NHID], bf16, tag="hw1")
            for half in range(vt_n // 4):
                xt_t = streamp.tile([NFV, 4 * 128], f32, tag="xt")
                nc.sync.dma_start(xt_t[:], xt[:, half * 512:(half + 1) * 512])
                for q in range(4):
                    vt = half * 4 + q
                    ps = multps.tile([128, NHID], f32, tag="mm")
                    nc.tensor.matmul(ps[:], xt_t[:, q * 128:(q + 1) * 128], w1_t[:],
                                     start=True, stop=True)
                    nc.vector.tensor_copy(hw1[:, vt * NHID:(vt + 1) * NHID], ps[:])
            # own-shard Hw1 (for the diagonal term; SPMD-safe via xt_sh input)
            xts_t = streamp.tile([NFV, iv], f32, tag="xt")
            nc.sync.dma_start(xts_t[:], xt_sh[:])
            hw1s = hwp.tile([128, ivt * NHID], bf16, tag="hw1s")
            for it in range(ivt):
                ps = multps.tile([128, NHID], f32, tag="mm")
                nc.tensor.matmul(ps[:], xts_t[:, it * 128:(it + 1) * 128], w1_t[:],
                                 start=True, stop=True)
                nc.vector.tensor_copy(hw1s[:, it * NHID:(it + 1) * NHID], ps[:])

            # ---------- d1 = Z @ p1  ([128, et_n] per-partition scalars) ----------
            d1_t = smallp.tile([128, et_n], f32, tag="d1")
            for q in range(et_n // 16):
                zch = streamp.tile([128, 16 * NFE], f32, tag="zch")
                nc.sync.dma_start(
                    zch[:].rearrange("p (t f) -> p t f", f=NFE),
                    zin[:].rearrange("(t p) f -> p t f", p=128)[:, q * 16:(q + 1) * 16, :],
                )
                nc.vector.tensor_mul(zch[:], zch[:], p1rep_t[:])
                nc.vector.reduce_sum(
                    d1_t[:, q * 16:(q + 1) * 16],
                    zch[:].rearrange("p (t f) -> p t f", f=NFE), axis=X_AX)

            # ---------- layer-1 moving operand: d1-scaled Tt[:, i-shard] ----------
            mov1 = movp.tile([128, et_n * iv], bf16, tag="mov")
            _c1 = et_n // 4
            for q in range(4):
                nc.sync.dma_start(
                    mov1[:].rearrange("p (t i) -> p t i", i=iv)
                    [:, q * _c1:(q + 1) * _c1, :],
                    tt_mov[:].rearrange("(t p) i -> p t i", p=128)
                    [:, q * _c1:(q + 1) * _c1, :],
                )
                for t in range(q * _c1, (q + 1) * _c1):
                    nc.vector.tensor_scalar_mul(
                        mov1[:, t * iv:(t + 1) * iv], mov1[:, t * iv:(t + 1) * iv],
                        1.0 if repeat > 1 else d1_t[:, t:t + 1],
                    )

            # ---------- layer 1: P = M1[j, i-shard] blocks, mask, SpMM ----------
            run_l1 = stage >= 2
            o_ps = [accps.tile([128, NHID], f32, tag="acc", name=f"o_ps{i}") for i in range(ivt)]
            kt1 = ne // 128                      # contraction tiles (64)
            slab_pe = None
            if diag == 'pe' and run_l1:
                slab_pe = slabp.tile([128, kt1 * 256], bf16, tag="slab", name="slab_pe")
                nc.sync.dma_start(
                    slab_pe[:].rearrange("p (t j) -> p t j", j=256),
                    tt_full[:].rearrange("(t p) j -> p t j", p=128)[:, :, 0:256],
                )
                adv_pe = adjp.tile([128, iv], f32, tag="adj", name="adv_pe")
                nc.sync.dma_start(adv_pe[:], advt[0:128, :])
            pend1 = None
            for js in range(vt_n // 2 if run_l1 else 0):          # slabs of 2 j-tiles
                if diag == 'pe':
                    slab = slab_pe
                else:
                    slab = slabp.tile([128, kt1 * 256], bf16, tag="slab")
                    nc.sync.dma_start(
                        slab[:].rearrange("p (t j) -> p t j", j=256),
                        tt_full[:].rearrange("(t p) j -> p t j", p=128)
                        [:, :, js * 256:(js + 1) * 256],
                    )
                for jj in range(2):
                    jt = js * 2 + jj
                    if diag == 'pe':
                        advt_t = adv_pe
                    else:
                        advt_t = adjp.tile([128, iv], f32, tag="adj")
                        nc.sync.dma_start(advt_t[:], advt[jt * 128:(jt + 1) * 128, :])
                    if diag == 'dma':
                        continue
                    p_ps = multps.tile([128, iv], f32, tag="mm")
                    for t in range(kt1):
                        nc.tensor.matmul(
                            p_ps[:],
                            slab[:, t * 256 + jj * 128: t * 256 + jj * 128 + 128],
                            mov1[:, t * iv:(t + 1) * iv],
                            start=(t == 0), stop=(t == kt1 - 1),
                        )
                    if pend1 is not None:
                        pa, pj = pend1
                        for it in range(ivt):
                            nc.tensor.matmul(
                                o_ps[it][:],
                                pa[:, it * 128:(it + 1) * 128],
                                hw1[:, pj * NHID:(pj + 1) * NHID],
                                start=(pj == 0), stop=False,
                                skip_group_check=True,
                            )
                    a_t = abufp.tile([128, iv], bf16, tag="a")
                    nc.vector.tensor_mul(a_t[:], p_ps[:], advt_t[:])
                    pend1 = (a_t, jt)

            if pend1 is not None:
                pa, pj = pend1
                for it in range(ivt):
                    nc.tensor.matmul(
                        o_ps[it][:],
                        pa[:, it * 128:(it + 1) * 128],
                        hw1[:, pj * NHID:(pj + 1) * NHID],
                        start=(pj == 0), stop=True,
                        skip_group_check=True,
                    )

            # ---------- Hw2 = relu(Z) @ W2  (bf16, [e-part tiles, c]) ----------
            # Issued AFTER layer 1 in program order so its ~20us of PE work
            # fills the d2-AllGather barrier window instead of delaying L1.
            hw2 = hwp.tile([128, et_n * NCLASS], bf16, tag="hw2")
            for ch in range(et_n // 4):
                ztc = streamp.tile([NFE, 4 * 128], f32, tag="zt")
                nc.sync.dma_start(ztc[:], ztf[:, ch * 512:(ch + 1) * 512])
                ztr = streamp.tile([NFE, 4 * 128], f32, tag="ztr")
                nc.scalar.activation(ztr[:], ztc[:], ACT.Relu)
                for q in range(4):
                    et = ch * 4 + q
                    ps = multps.tile([128, NCLASS], f32, tag="mm")
                    nc.tensor.matmul(ps[:], ztr[:, q * 128:(q + 1) * 128], w2_t[:],
                                     start=True, stop=True)
                    nc.vector.tensor_copy(hw2[:, et * NCLASS:(et + 1) * NCLASS], ps[:])
            # own-shard HwS = (relu(Z) @ W2).T  [c, i]  f32, diag-premultiplied
            hwsd = hwp.tile([NCLASS, ie], f32, tag="hwsd")
            for hh in range(ieh):
                zs_t = streamp.tile([NFE, 512], f32, tag="zt")
                nc.sync.dma_start(zs_t[:], zt_sh[:, hh * 512:(hh + 1) * 512])
                zsr = streamp.tile([NFE, 512], f32, tag="ztr")
                nc.scalar.activation(zsr[:], zs_t[:], ACT.Relu)
                psS = multps.tile([NCLASS, 512], f32, tag="mm")
                nc.tensor.matmul(psS[:], w2_t[:], zsr[:],
                                 start=True, stop=True)
                # premultiply by diag(adj_e) shard while evacuating
                nc.vector.tensor_mul(
                    hwsd[:, hh * 512:(hh + 1) * 512], psS[:],
                    aedb_t[:, hh * 512:(hh + 1) * 512],
                )

            # ---------- X1 = relu(O + diag*Hw1s + b1), d2 shard ----------
            d2sh = smallp.tile([128, ivt], f32, tag="d2sh")
            junkh = smallp.tile([128, NHID], f32, tag="junkh")
            if stage < 3:
                nc.gpsimd.memset(d2sh[:], 1.0)
            for it in range(ivt if stage >= 3 else 0):
                x1d = x1p.tile([128, NHID], f32, tag="x1d")
                nc.vector.tensor_scalar_mul(
                    x1d[:], hw1s[:, it * NHID:(it + 1) * NHID], avd_t[:, it:it + 1])
                x1a = x1p.tile([128, NHID], f32, tag="x1a")
                nc.vector.tensor_add(x1a[:], x1d[:], o_ps[it][:])
                x1b = x1p.tile([128, NHID], f32, tag="x1b")
                nc.vector.tensor_add(x1b[:], x1a[:], b1b_t[:])
                x1c = x1p.tile([128, NHID], f32, tag="x1c")
                nc.scalar.activation(x1c[:], x1b[:], ACT.Relu)
                nc.vector.tensor_mul(junkh[:], x1c[:], p2b_t[:])
                nc.vector.reduce_sum(d2sh[:, it:it + 1], junkh[:], axis=X_AX)

            # ---------- AllGather d2 ----------
            d2_in = dramp.tile([iv, 1], f32, tag="d2in")
            nc.sync.dma_start(
                d2_in[:].rearrange("(t p) o -> p (t o)", p=128), d2sh[:]
            )
            d2_out = dramp.tile([nv, 1], f32, tag="d2out")
            d2a = smallp.tile([128, vt_n], f32, tag="d2a")
            if COLL_MODE >= 1:
                nc.gpsimd.collective_compute(
                    "AllGather", mybir.AluOpType.bypass,
                    replica_groups=[list(range(n_cores))],
                    ins=[d2_in.opt()], outs=[d2_out.opt()],
                )
                nc.sync.dma_start(
                    d2a[:], d2_out[:].rearrange("(t p) o -> p (t o)", p=128)
                )
            else:
                nc.gpsimd.memset(d2a[:], 1.0)

            # ---------- layer-2 moving operand: d2-scaled T[:, i-shard] ----------
            mov2 = movp.tile([128, vt_n * ie], bf16, tag="mov")
            _c2 = vt_n // 4
            for q in range(4 if stage >= 4 else 0):
                nc.sync.dma_start(
                    mov2[:].rearrange("p (t i) -> p t i", i=ie)
                    [:, q * _c2:(q + 1) * _c2, :],
                    t_mov[:].rearrange("(t p) i -> p t i", p=128)
                    [:, q * _c2:(q + 1) * _c2, :],
                )
                for t in range(q * _c2, (q + 1) * _c2):
                    nc.vector.tensor_scalar_mul(
                        mov2[:, t * ie:(t + 1) * ie], mov2[:, t * ie:(t + 1) * ie],
                        1.0 if repeat > 1 else d2a[:, t:t + 1],
                    )

            # ---------- layer 2: P = M2[j, i-shard] blocks, mask, SpMM ----------
            if stage < 5:
                a2dummy = abufp.tile([128, ie], bf16, tag="a")
                nc.gpsimd.memset(a2dummy[:], 0.0)
            z_ps = [accps.tile([NCLASS, 512], f32, tag="acc", name=f"z_ps{i}") for i in range(ieh)]
            kt2 = nv // 128                      # contraction tiles (32)
            run_l2 = stage >= 5
            if not run_l2:
                for hh in range(ieh):
                    nc.tensor.matmul(z_ps[hh][:], hw2[:, 0:NCLASS], a2dummy[:, hh * 512:(hh + 1) * 512], start=True, stop=True)
            pend2 = [None] * ieh
            slab2_pe = None
            if diag == 'pe' and run_l2:
                slab2_pe = slabp.tile([128, kt2 * 256], bf16, tag="slab", name="slab2_pe")
                nc.sync.dma_start(
                    slab2_pe[:].rearrange("p (t j) -> p t j", j=256),
                    t_full[:].rearrange("(t p) j -> p t j", p=128)[:, :, 0:256],
                )
                aev_pe = adjp.tile([128, ie], f32, tag="adj", name="aev_pe")
                nc.sync.dma_start(aev_pe[:], aevt[0:128, :])
            for js in range(et_n // 2 if run_l2 else 0):
                if diag == 'pe':
                    slab2 = slab2_pe
                else:
                    slab2 = slabp.tile([128, kt2 * 256], bf16, tag="slab")
                    nc.sync.dma_start(
                        slab2[:].rearrange("p (t j) -> p t j", j=256),
                        t_full[:].rearrange("(t p) j -> p t j", p=128)
                        [:, :, js * 256:(js + 1) * 256],
                    )
                for jj in range(2):
                    jt = js * 2 + jj
                    if diag == 'pe':
                        aevt_t = aev_pe
                    else:
                        aevt_t = adjp.tile([128, ie], f32, tag="adj")
                        nc.sync.dma_start(aevt_t[:], aevt[jt * 128:(jt + 1) * 128, :])
                    if diag == 'dma':
                        continue
                    a2_t = abufp.tile([128, ie], bf16, tag="a")
                    for hh in range(ieh):
                        p_ps = multps.tile([128, 512], f32, tag="mm")
                        for t in range(kt2):
                            nc.tensor.matmul(
                                p_ps[:],
                                slab2[:, t * 256 + jj * 128: t * 256 + jj * 128 + 128],
                                mov2[:, t * ie + hh * 512: t * ie + hh * 512 + 512],
                                start=(t == 0), stop=(t == kt2 - 1),
                            )
                        if pend2[hh] is not None:
                            pa, pj = pend2[hh]
                            nc.tensor.matmul(
                                z_ps[hh][:],
                                hw2[:, pj * NCLASS:(pj + 1) * NCLASS],
                                pa[:, hh * 512:(hh + 1) * 512],
                                start=(pj == 0), stop=False,
                                skip_group_check=True,
                            )
                        nc.vector.tensor_mul(
                            a2_t[:, hh * 512:(hh + 1) * 512], p_ps[:],
                            aevt_t[:, hh * 512:(hh + 1) * 512],
                        )
                        pend2[hh] = (a2_t, jt)

            for hh in range(ieh):
                if pend2[hh] is not None:
                    pa, pj = pend2[hh]
                    nc.tensor.matmul(
                        z_ps[hh][:],
                        hw2[:, pj * NCLASS:(pj + 1) * NCLASS],
                        pa[:, hh * 512:(hh + 1) * 512],
                        start=(pj == 0), stop=True,
                        skip_group_check=True,
                    )

            # ---------- epilogue: bias + diag + log_softmax ----------
            zb = zbp.tile([NCLASS, ie], f32, tag="zb")
            for hh in range(ieh):
                nc.vector.tensor_add(
                    zb[:, hh * 512:(hh + 1) * 512], z_ps[hh][:],
                    hwsd[:, hh * 512:(hh + 1) * 512],
                )
            nc.vector.tensor_scalar_add(zb[:], zb[:], b2_t[:])

            mloc = smallp.tile([NCLASS, 1], f32, tag="mloc")
            nc.vector.reduce_max(mloc[:], zb[:], axis=X_AX)
            mx_in = dramp.tile([NCLASS, 1], f32, tag="mxin")
            nc.sync.dma_start(mx_in[:], mloc[:])
            mx_out = dramp.tile([NCLASS, 1], f32, tag="mxout")
            mglob = smallp.tile([NCLASS, 1], f32, tag="mglob")
            if COLL_MODE >= 2:
                nc.gpsimd.collective_compute(
                    "AllReduce", mybir.AluOpType.max,
                    replica_groups=[list(range(n_cores))],
                    ins=[mx_in.opt()], outs=[mx_out.opt()],
                )
                nc.sync.dma_start(mglob[:], mx_out[:])
            else:
                nc.vector.tensor_copy(mglob[:], mloc[:])
            negm = smallp.tile([NCLASS, 1], f32, tag="negm")
            nc.vector.tensor_scalar_mul(negm[:], mglob[:], -1.0)
            ejunk = zbp.tile([NCLASS, ie], f32, tag="zscr")
            sloc = smallp.tile([NCLASS, 1], f32, tag="sloc")
            nc.scalar.activation(ejunk[:], zb[:], ACT.Exp, bias=negm[:],
                                 scale=1.0, accum_out=sloc[:])
            sm_in = dramp.tile([NCLASS, 1], f32, tag="smin")
            nc.sync.dma_start(sm_in[:], sloc[:])
            sm_out = dramp.tile([NCLASS, 1], f32, tag="smout")
            sglob = smallp.tile([NCLASS, 1], f32, tag="sglob")
            if COLL_MODE >= 2:
                nc.gpsimd.collective_compute(
                    "AllReduce", mybir.AluOpType.add,
                    replica_groups=[list(range(n_cores))],
                    ins=[sm_in.opt()], outs=[sm_out.opt()],
                )
                nc.sync.dma_start(sglob[:], sm_out[:])
            else:
                nc.vector.tensor_copy(sglob[:], sloc[:])
            lns = smallp.tile([NCLASS, 1], f32, tag="lns")
            nc.scalar.activation(lns[:], sglob[:], ACT.Ln)
            lse = smallp.tile([NCLASS, 1], f32, tag="lse")
            nc.vector.tensor_add(lse[:], mglob[:], lns[:])
            out_sb = zbp.tile([NCLASS, ie], f32, tag="zscr")
            nc.vector.tensor_scalar_sub(out_sb[:], zb[:], lse[:])
            nc.sync.dma_start(zt_out[:], out_sb[:])

    nc.compile()
    return nc


def prepare_inputs(inputs, nv=NV, ne=NE, n_cores=N_CORES):
    """Host-side layout prep: transposes, bf16 casts, shard extraction."""
    f32 = np.float32
    X = np.asarray(inputs["X"], f32)
    Z = np.asarray(inputs["Z"], f32)
    adj_e = np.asarray(inputs["adj_e"], f32)
    adj_v = np.asarray(inputs["adj_v"], f32)
    T = np.asarray(inputs["T"], f32)
    W1 = np.asarray(inputs["W1"], f32)
    b1 = np.asarray(inputs["b1"], f32)
    p1 = np.asarray(inputs["p1"], f32)
    W2 = np.asarray(inputs["W2"], f32)
    b2 = np.asarray(inputs["b2"], f32)
    p2 = np.asarray(inputs["p2"], f32)

    iv, ie = nv // n_cores, ne // n_cores
    T_bf = T.astype(BF16)
    t_full = np.ascontiguousarray(T_bf)
    tt_full = np.ascontiguousarray(T_bf.T)
    xt = np.ascontiguousarray(X.T)
    ztf = np.ascontiguousarray(Z.T)
    av_d = np.ascontiguousarray(np.diagonal(adj_v))
    ae_d = np.ascontiguousarray(np.diagonal(adj_e))
    b1b = np.ascontiguousarray(np.tile(b1[None, :], (128, 1)))
    p1b = np.ascontiguousarray(np.tile(p1[None, :], (128, 1)))
    p2b = np.ascontiguousarray(np.tile(p2[None, :], (128, 1)))
    b2c = np.ascontiguousarray(b2.reshape(NCLASS, 1))

    in_maps = []
    for c in range(n_cores):
        vs = slice(c * iv, (c + 1) * iv)
        es = slice(c * ie, (c + 1) * ie)
        advt = np.ascontiguousarray(adj_v[vs, :].T)
        advt[np.arange(iv) + c * iv, np.arange(iv)] = 0.0
        aevt = np.ascontiguousarray(adj_e[es, :].T)
        aevt[np.arange(ie) + c * ie, np.arange(ie)] = 0.0
        in_maps.append({
            "tt_full": tt_full,
            "t_full": t_full,
            "tt_mov": np.ascontiguousarray(tt_full[:, vs]),
            "t_mov": np.ascontiguousarray(t_full[:, es]),
            "advt": advt,
            "aevt": aevt,
            "avdiag": np.ascontiguousarray(av_d[vs].reshape(iv, 1)),
            "aedb": np.ascontiguousarray(np.tile(ae_d[es][None, :], (NCLASS, 1))),
            "xt": xt,
            "xt_sh": np.ascontiguousarray(xt[:, vs]),
            "w1": W1,
            "b1b": b1b,
            "zin": Z,
            "p1b": p1b,
            "p2b": p2b,
            "ztf": ztf,
            "zt_sh": np.ascontiguousarray(ztf[:, es]),
            "w2": W2,
            "b2c": b2c,
        })
    return in_maps


_PROGRAM_CACHE = {}


def _get_program(nv=NV, ne=NE, n_cores=N_CORES):
    key = (nv, ne, n_cores)
    if key not in _PROGRAM_CACHE:
        _PROGRAM_CACHE[key] = build_program(nv, ne, n_cores)
    return _PROGRAM_CACHE[key]


def run(inputs, nv=NV, ne=NE, n_cores=N_CORES):
    from concourse.bass_utils import run_bass_kernel_spmd
    nc = _get_program(nv, ne, n_cores)
    in_maps = prepare_inputs(inputs, nv, ne, n_cores)
    res = run_bass_kernel_spmd(nc, in_maps, core_ids=list(range(n_cores)))
    zt = np.concatenate([res.results[c]["zt_out"] for c in range(n_cores)], axis=1)
    return np.ascontiguousarray(zt.T.astype(np.float32))


def kernel(**inputs) -> np.ndarray:
    return run(inputs, NV, NE, N_CORES)

